# revision 14
# baseline (speedup 1.0000x reference)
"""Trainium2 Bass kernel for nn_LinearDiffusion (truncated Taylor expm(a) @ x).

Math: a = row-normalized symmetric scatter of per-head edge weights onto an
(H, N, N) zero tensor; result = sum_{i=0..6} a^i x / i! with x = h reshaped
per-head.

Strategy (8 NeuronCores, one chip) — v5, TensorE-gather, no spill:
  * x (8192 x 64 fp16, all heads together) lives in SBUF on every core; the
    per-edge gather x[src] is computed by TensorE one-hot matmuls from the
    SBUF-resident copy (the v1 bottleneck was per-edge DMA gather on GpSimd).
  * Core k owns dst rows [k*1024, (k+1)*1024) = 8 blocks of 128.  Edges are
    binned per (dst block jb, src block sb) cell.  Cell capacities are
    data-adaptive: cap = max-over-cores count, rounded up to 32 (the PE
    column-group granularity), so ALL edges fit — no overflow path.  The
    cell grid (shared by all cores; only table data differs) tiles each dst
    block's slot space into 128-slot chunks:
      - gather:  per piece (cell range clipped to chunk + base-alignment
                 rules), one matmul  psum[base:base+w, chunk] =
                 gmat_cols^T @ xsb[:, sb, :]   (gmat: fp8 one-hot of
                 src_local, zero-padded; every PSUM byte written, start=True)
      - weights: one DVE multiply per 8-chunk PSUM bank with the per-head
                 w4 table (broadcast over the 16 feats of each head), fp16 out
      - scatter: per chunk, one matmul into the iteration's output bank
                 pout[:, jb*64:+64] += sca[:, chunk]^T @ xgw  (sca: fp8
                 one-hot of dst_local; PSUM accumulation opened by a
                 full-bank zero matmul — hw clears has_written per element)
  * Between iterations: AllGather of the fp16 x slices (128 KB/rank) and a
    single strided DMA reload of xsb.  Output accumulates in fp32.
"""

import math
import os
from dataclasses import dataclass

import numpy as np

# Small AllGathers hit the RDH algorithm; try mesh (set before NRT loads).
os.environ.setdefault("NEURON_RT_DBG_RDH_CC", "0")

import concourse.bass as bass  # noqa: F401  (kept for callers)
import concourse.tile as tile
from concourse import bacc, mybir
from concourse.bass_utils import run_bass_kernel_spmd

# ----------------------------------------------------------------- config

N, H, E, D = 8192, 4, 131072, 64
d = D // H
NCORES = 8
BLK = 128                  # dst-block size == PSUM partition width
NSB = N // BLK             # src blocks (64)
JBS = N // NCORES // BLK   # dst blocks per core (8)
K_TAYLOR = 6
GRP = 8                    # chunks per PSUM gather bank


@dataclass(frozen=True)
class Cfg:
    n: int = N
    n_cores: int = NCORES
    kt: int = K_TAYLOR  # Taylor depth (debug)


# ----------------------------------------------------------- preprocessing


def _entries(e, src, dst, n):
    """Unique symmetric entries with 'last write wins' duplicate semantics,
    matching jax's .at[].set() on CPU. Returns (rows, cols, w[H, nnz])."""
    src = src.astype(np.int64)
    dst = dst.astype(np.int64)
    n_edges = len(src)
    keys = np.concatenate([src * n + dst, dst * n + src])
    eid = np.concatenate([np.arange(n_edges), np.arange(n_edges)])
    order = np.arange(2 * n_edges)
    perm = np.lexsort((-order, keys))
    k_sorted = keys[perm]
    first = np.ones(len(k_sorted), dtype=bool)
    first[1:] = k_sorted[1:] != k_sorted[:-1]
    win = perm[first]
    ukeys = k_sorted[first]
    rows = (ukeys // n).astype(np.int64)
    cols = (ukeys % n).astype(np.int64)
    weids = eid[win]
    vals = e[:, weids].astype(np.float64)  # (H, nnz)
    nheads = e.shape[0]
    rowsum = np.zeros((nheads, n), dtype=np.float64)
    for hh in range(nheads):
        rowsum[hh] = np.bincount(rows, weights=vals[hh], minlength=n)
    w = (vals / rowsum[:, rows]).astype(np.float32)
    return rows, cols, w


def _split_piece(a, b):
    """Split slot range [a, b) into matmul-legal (base, width) pieces:
    within a 128-chunk, base 0 allows w<=128, base 64 allows w<=64,
    bases 32/96 allow w<=32."""
    out = []
    while a < b:
        base = a % 128
        if base == 0:
            w = min(b - a, 128)
        elif base % 64 == 0:
            w = min(b - a, 64)
        else:
            w = min(b - a, 32)
        # don't cross the chunk boundary
        w = min(w, 128 - base)
        out.append((a, w))
        a += w
    return out


def _make_plan(rows, cols, cfg: Cfg):
    """Shared (cross-core) cell grid. Returns plan dict."""
    rpc = cfg.n // cfg.n_cores
    cnt = np.zeros((cfg.n_cores, JBS, NSB), dtype=np.int64)
    for k in range(cfg.n_cores):
        m = (rows >= k * rpc) & (rows < (k + 1) * rpc)
        r = rows[m] - k * rpc
        cell = (r // BLK) * NSB + (cols[m] // BLK)
        cnt[k] = np.bincount(cell, minlength=JBS * NSB).reshape(JBS, NSB)
    caps = ((cnt.max(axis=0) + 31) // 32 * 32).astype(np.int64)  # [JBS, NSB]
    jbs = []
    for j in range(JBS):
        cj = caps[j].copy()
        tot = int(cj.sum())
        pad = (-tot) % 128
        cj[NSB - 1] += pad          # bump last cell so chunks tile exactly
        starts = np.concatenate([[0], np.cumsum(cj)])
        S = int(starts[-1])
        nch = S // 128
        # pieces per chunk: (sb, gcol_start, base, width)
        pieces = [[] for _ in range(nch)]
        for sb in range(NSB):
            a, b = int(starts[sb]), int(starts[sb + 1])
            for (s0, w) in _split_piece(a, b):
                pieces[s0 // 128].append((sb, s0, s0 % 128, w))
        jbs.append({
            "caps": cj, "starts": starts, "S": S, "nch": nch,
            "pieces": pieces,
        })
    nch_tot = sum(jb["nch"] for jb in jbs)
    chunk_base = np.concatenate([[0], np.cumsum([jb["nch"] for jb in jbs])])
    return {"jbs": jbs, "nch_tot": int(nch_tot), "chunk_base": chunk_base,
            "key": tuple(int(x) for x in caps.reshape(-1))}


def _make_tables(e, src, dst, cfg: Cfg):
    """Per-core device tables + shared plan."""
    import ml_dtypes

    n = cfg.n
    rows, cols, w = _entries(e, src, dst, n)
    rpc = n // cfg.n_cores
    plan = _make_plan(rows, cols, cfg)
    jbs = plan["jbs"]
    nch_tot = plan["nch_tot"]
    cbase = plan["chunk_base"]
    S_tot = sum(jb["S"] for jb in jbs)
    s_base = np.concatenate([[0], np.cumsum([jb["S"] for jb in jbs])])

    tables = []
    for k in range(cfg.n_cores):
        m = (rows >= k * rpc) & (rows < (k + 1) * rpc)
        r = rows[m] - k * rpc
        c = cols[m]
        wv = w[:, m]
        jb = r // BLK
        dl = r % BLK
        sb = c // BLK
        sl = c % BLK
        o = np.lexsort((sb, jb))
        jb, dl, sb, sl, wv = jb[o], dl[o], sb[o], sl[o], wv[:, o]
        cell = jb * NSB + sb
        ic = np.arange(len(cell)) - np.searchsorted(cell, cell)

        gmat = np.zeros((128, S_tot), dtype=ml_dtypes.float8_e4m3fn)
        sca = np.zeros((128, nch_tot * 128), dtype=ml_dtypes.float8_e4m3fn)
        w4m = np.zeros((128, nch_tot, H), dtype=np.float32)

        starts_per_jb = np.stack([jbs[j]["starts"][:-1] for j in range(JBS)])
        slot = starts_per_jb[jb, sb] + ic           # slot within jb
        gcol = s_base[jb] + slot
        chunk = cbase[jb] + slot // 128
        p = slot % 128
        gmat[sl, gcol] = 1.0
        sca[p, chunk * 128 + dl] = 1.0
        w4m[p, chunk, :] = wv.T

        tables.append({
            "gmat": np.ascontiguousarray(gmat),
            "sca": np.ascontiguousarray(sca),
            "w4m": np.ascontiguousarray(w4m.reshape(128, -1)),
        })
    return tables, plan


def _emulate(tables, plan, xe, cfg: Cfg):
    """Numpy emulation of one iteration y = A @ x using tables + plan."""
    jbs = plan["jbs"]
    cbase = plan["chunk_base"]
    s_base = np.concatenate([[0], np.cumsum([jb["S"] for jb in jbs])])
    out = np.zeros((NCORES, 128, JBS, 64), dtype=np.float32)
    xsb = (
        xe.reshape(NCORES, 128, JBS, 64).transpose(1, 0, 2, 3)
        .reshape(128, NSB, 64).astype(np.float32)
    )
    for k in range(cfg.n_cores):
        t = tables[k]
        gmat = t["gmat"].astype(np.float32)
        sca = t["sca"].astype(np.float32)
        w4m = t["w4m"].reshape(128, -1, H)
        for j in range(JBS):
            acc = np.zeros((128, 64), dtype=np.float32)
            for ci in range(jbs[j]["nch"]):
                cg = cbase[j] + ci
                pg = np.zeros((128, 64), dtype=np.float32)
                for (sb, s0, base, wd) in jbs[j]["pieces"][ci]:
                    g = gmat[:, s_base[j] + s0: s_base[j] + s0 + wd]
                    pg[base:base + wd] = g.T @ xsb[:, sb, :]
                w4 = w4m[:, cg, :]
                xgw = (pg.reshape(128, H, d)
                       * w4[:, :, None]).reshape(128, 64).astype(np.float16)
                s = sca[:, cg * 128:(cg + 1) * 128]
                acc += s.T @ xgw.astype(np.float32)
            out[k, :, j, :] = acc
    return out.transpose(0, 2, 1, 3).reshape(N, 64)


# ------------------------------------------------------------ bass program

_FP32 = mybir.dt.float32
_FP16 = mybir.dt.float16
_FP8 = mybir.dt.float8e4


def _build_program(cfg: Cfg, plan):
    kt = cfg.kt
    jbs = plan["jbs"]
    nch_tot = plan["nch_tot"]
    cbase = plan["chunk_base"]
    s_base = np.concatenate([[0], np.cumsum([jb["S"] for jb in jbs])])
    S_tot = int(s_base[-1])
    nc = bacc.Bacc(
        "TRN2",
        target_bir_lowering=False,
        debug=False,
        num_devices=cfg.n_cores,
    )

    xe0_d = nc.dram_tensor("xe0", [1024, 512], _FP16, kind="ExternalInput").ap()
    x0s_d = nc.dram_tensor("x0s", [128, 512], _FP32, kind="ExternalInput").ap()
    gmat_d = nc.dram_tensor("gmat", [128, S_tot], _FP8,
                            kind="ExternalInput").ap()
    sca_d = nc.dram_tensor("sca", [128, nch_tot * 128], _FP8,
                           kind="ExternalInput").ap()
    w4m_d = nc.dram_tensor("w4m", [128, nch_tot * H], _FP32,
                           kind="ExternalInput").ap()
    out_d = nc.dram_tensor("out", [128, 512], _FP32, kind="ExternalOutput").ap()

    slice_in = nc.dram_tensor("slice_in", [128, 512], _FP16).ap()
    xallE = nc.dram_tensor("xallE", [1024, 512], _FP16,
                           addr_space="Shared").ap()

    groups = [list(range(cfg.n_cores))]

    with tile.TileContext(nc) as tc:
        with (
            tc.tile_pool(name="tables", bufs=1) as tp,
            tc.tile_pool(name="xgw", bufs=3) as xgwp,
            tc.tile_pool(name="psg", bufs=3, space="PSUM") as pgp,
            tc.tile_pool(name="pso", bufs=2, space="PSUM") as pop,
        ):
            w4m_sb = tp.tile([128, nch_tot, H], _FP32)
            xsb = tp.tile([128, NSB, 64], _FP16)
            xnext = tp.tile([128, JBS * 64], _FP16)
            result = tp.tile([128, JBS * 64], _FP32)
            z128 = tp.tile([128, 128], _FP8)
            gmat_sb = [tp.tile([128, jbs[j]["S"]], _FP8, tag=f"gm{j}",
                               name=f"gmat_sb{j}")
                       for j in range(JBS)]
            sca_sb = [tp.tile([128, jbs[j]["nch"] * 128], _FP8, tag=f"sc{j}",
                              name=f"sca_sb{j}")
                      for j in range(JBS)]

            nc.vector.memset(z128[:], 0.0)
            # small tables + x first so jb0 compute starts early
            nc.sync.dma_start(
                out=w4m_sb[:].rearrange("p c h -> p (c h)"), in_=w4m_d)
            nc.sync.dma_start(
                out=xsb[:].rearrange("p (k j) f -> p k (j f)", k=NCORES),
                in_=xe0_d.rearrange("(k p) f -> p k f", p=128),
            )
            nc.sync.dma_start(out=result[:], in_=x0s_d)
            for j in range(JBS):
                nc.sync.dma_start(
                    out=gmat_sb[j][:],
                    in_=gmat_d[:, int(s_base[j]):int(s_base[j + 1])])
                nc.sync.dma_start(
                    out=sca_sb[j][:],
                    in_=sca_d[:, int(cbase[j]) * 128:int(cbase[j + 1]) * 128])

            for it in range(1, kt + 1):
                coef = 1.0 / math.factorial(it)
                pout = pop.tile([128, JBS * 64], _FP32, tag="pout")
                # open the accumulation bank: zero matmul writes every byte
                nc.tensor.matmul(
                    pout[:],
                    lhsT=z128[:],
                    rhs=xsb[:, 0:8, :],
                    start=True,
                    stop=False,
                )
                for j in range(JBS):
                    nch_j = jbs[j]["nch"]
                    ngr = -(-nch_j // GRP)
                    for g in range(ngr):
                        c0 = g * GRP
                        c1 = min(nch_j, c0 + GRP)
                        gsz = c1 - c0
                        pg = pgp.tile([128, GRP * 64], _FP32, tag="pg")
                        for ci in range(c0, c1):
                            for (sb, s0, base, wd) in jbs[j]["pieces"][ci]:
                                nc.tensor.matmul(
                                    pg[base:base + wd,
                                       (ci - c0) * 64:(ci - c0 + 1) * 64],
                                    lhsT=gmat_sb[j][:, s0:s0 + wd],
                                    rhs=xsb[:, sb, :],
                                    start=True,
                                    stop=True,
                                    tile_position=(
                                        (0, 96) if base == 96 else None),
                                )
                        xgw = xgwp.tile([128, GRP, 64], _FP16, tag="xgw")
                        pg4 = pg[:, 0:gsz * 64].rearrange(
                            "p (c h f) -> p c h f", c=gsz, h=H)
                        wv = (
                            w4m_sb[:, int(cbase[j]) + c0:
                                   int(cbase[j]) + c1, :]
                            .unsqueeze(3)
                            .to_broadcast([128, gsz, H, d])
                        )
                        nc.vector.tensor_mul(
                            xgw[:, 0:gsz, :].rearrange(
                                "p c (h f) -> p c h f", h=H),
                            pg4, wv)
                        last_mm = (j == JBS - 1 and c1 == nch_j)
                        for ci in range(c0, c1):
                            nc.tensor.matmul(
                                pout[:, j * 64:(j + 1) * 64],
                                lhsT=sca_sb[j][:, ci * 128:(ci + 1) * 128],
                                rhs=xgw[:, ci - c0, :],
                                start=False,
                                stop=(last_mm and ci == c1 - 1),
                            )
                # ---- evacuate + Taylor accumulate
                nc.scalar.copy(xnext[:], pout[:])
                nc.vector.scalar_tensor_tensor(
                    result[:],
                    pout[:],
                    coef,
                    result[:],
                    op0=mybir.AluOpType.mult,
                    op1=mybir.AluOpType.add,
                )
                if it < kt:
                    nc.sync.dma_start(out=slice_in, in_=xnext[:])
                    nc.gpsimd.collective_compute(
                        "AllGather",
                        mybir.AluOpType.bypass,
                        replica_groups=groups,
                        ins=[slice_in],
                        outs=[xallE],
                    )
                    nc.sync.dma_start(
                        out=xsb[:].rearrange(
                            "p (k j) f -> p k (j f)", k=NCORES),
                        in_=xallE.rearrange("(k p) f -> p k f", p=128),
                    )

            nc.sync.dma_start(out=out_d, in_=result[:])

    nc.compile()
    return nc


# ------------------------------------------------------------------ driver

_CACHE = {}


def _get_program(cfg: Cfg, plan):
    key = (cfg, plan["key"])
    if key not in _CACHE:
        _CACHE[key] = _build_program(cfg, plan)
    return _CACHE[key]


def _prep_x(h):
    """h [N, D] -> x0 node-major [N, D] (head-interleaved feats)."""
    return np.ascontiguousarray(
        h.reshape(H, N, d).transpose(1, 0, 2).reshape(N, D))


def _to_exchange(x0):
    """node-major [8192, 64] -> exchange layout [1024, 512]."""
    return np.ascontiguousarray(
        x0.reshape(NCORES, JBS, 128, 64).transpose(0, 2, 1, 3)
        .reshape(1024, 512))


def run(h, e, src, dst, cfg: Cfg = Cfg(), trace: bool = False):
    h = np.asarray(h, dtype=np.float32)
    e = np.asarray(e, dtype=np.float32)
    src = np.asarray(src)
    dst = np.asarray(dst)
    assert h.shape == (cfg.n, D) and e.shape == (H, E)

    tables, plan = _make_tables(e, src, dst, cfg)
    x0 = _prep_x(h)
    xe0 = _to_exchange(x0).astype(np.float16)
    in_maps = []
    for k in range(cfg.n_cores):
        x0s = np.ascontiguousarray(
            x0[k * 1024:(k + 1) * 1024]
            .reshape(JBS, 128, 64).transpose(1, 0, 2).reshape(128, 512))
        t = tables[k]
        in_maps.append(
            {
                "xe0": xe0,
                "x0s": x0s,
                "gmat": t["gmat"],
                "sca": t["sca"],
                "w4m": t["w4m"],
            }
        )
    nc = _get_program(cfg, plan)
    res = run_bass_kernel_spmd(
        nc, in_maps, list(range(cfg.n_cores)), trace=trace)
    out = np.stack([res.results[k]["out"] for k in range(cfg.n_cores)])
    out = (out.reshape(NCORES, 128, JBS, 64).transpose(0, 2, 1, 3)
           .reshape(N, 64))
    out = np.ascontiguousarray(
        out.reshape(N, H, d).transpose(1, 0, 2)).reshape(N, D)
    return out, res


def kernel(h, e, src, dst):
    out, _ = run(h, e, src, dst)
    return out


# revision 19
# speedup vs baseline: 2.0773x; 2.0773x over previous
"""Trainium2 Bass kernel for nn_LinearDiffusion (truncated Taylor expm(a) @ x).

Math: a = row-normalized symmetric scatter of per-head edge weights onto an
(H, N, N) zero tensor; result = sum_{i=0..6} a^i x / i! with x = h reshaped
per-head.

Strategy (8 NeuronCores, one chip) — v5, TensorE-gather, no spill:
  * x (8192 x 64 fp16, all heads together) lives in SBUF on every core; the
    per-edge gather x[src] is computed by TensorE one-hot matmuls from the
    SBUF-resident copy (the v1 bottleneck was per-edge DMA gather on GpSimd).
  * Core k owns dst rows [k*1024, (k+1)*1024) = 8 blocks of 128.  Edges are
    binned per (dst block jb, src block sb) cell.  Cell capacities are
    data-adaptive: cap = max-over-cores count, rounded up to 32 (the PE
    column-group granularity), so ALL edges fit — no overflow path.  The
    cell grid (shared by all cores; only table data differs) tiles each dst
    block's slot space into 128-slot chunks:
      - gather:  per piece (cell range clipped to chunk + base-alignment
                 rules), one matmul  psum[base:base+w, chunk] =
                 gmat_cols^T @ xsb[:, sb, :]   (gmat: fp8 one-hot of
                 src_local, zero-padded; every PSUM byte written, start=True)
      - weights: one DVE multiply per 8-chunk PSUM bank with the per-head
                 w4 table (broadcast over the 16 feats of each head), fp16 out
      - scatter: per chunk, one matmul into the iteration's output bank
                 pout[:, jb*64:+64] += sca[:, chunk]^T @ xgw  (sca: fp8
                 one-hot of dst_local; PSUM accumulation opened by a
                 full-bank zero matmul — hw clears has_written per element)
  * Between iterations: AllGather of the fp16 x slices (128 KB/rank) and a
    single strided DMA reload of xsb.  Output accumulates in fp32.
"""

import math
import os
from dataclasses import dataclass

import numpy as np

# Small AllGathers hit the RDH algorithm; try mesh (set before NRT loads).
os.environ.setdefault("NEURON_RT_DBG_RDH_CC", "0")

import concourse.bass as bass  # noqa: F401  (kept for callers)
import concourse.tile as tile
from concourse import bacc, mybir
from concourse.bass_utils import run_bass_kernel_spmd

# ----------------------------------------------------------------- config

N, H, E, D = 8192, 4, 131072, 64
d = D // H
NCORES = 8
BLK = 128                  # dst-block size == PSUM partition width
NSB = N // BLK             # src blocks (64)
JBS = N // NCORES // BLK   # dst blocks per core (8)
K_TAYLOR = 6
GRP = 8                    # chunks per PSUM gather bank


@dataclass(frozen=True)
class Cfg:
    n: int = N
    n_cores: int = NCORES
    kt: int = K_TAYLOR  # Taylor depth (debug)


# ----------------------------------------------------------- preprocessing


def _entries(e, src, dst, n):
    """Unique symmetric entries with 'last write wins' duplicate semantics,
    matching jax's .at[].set() on CPU. Returns (rows, cols, w[H, nnz])."""
    src = src.astype(np.int64)
    dst = dst.astype(np.int64)
    n_edges = len(src)
    keys = np.concatenate([src * n + dst, dst * n + src])
    eid = np.concatenate([np.arange(n_edges), np.arange(n_edges)])
    order = np.arange(2 * n_edges)
    perm = np.lexsort((-order, keys))
    k_sorted = keys[perm]
    first = np.ones(len(k_sorted), dtype=bool)
    first[1:] = k_sorted[1:] != k_sorted[:-1]
    win = perm[first]
    ukeys = k_sorted[first]
    rows = (ukeys // n).astype(np.int64)
    cols = (ukeys % n).astype(np.int64)
    weids = eid[win]
    vals = e[:, weids].astype(np.float64)  # (H, nnz)
    nheads = e.shape[0]
    rowsum = np.zeros((nheads, n), dtype=np.float64)
    for hh in range(nheads):
        rowsum[hh] = np.bincount(rows, weights=vals[hh], minlength=n)
    w = (vals / rowsum[:, rows]).astype(np.float32)
    return rows, cols, w


def _make_plan(rows, cols, cfg: Cfg):
    """Shared (cross-core) cell grid.  Each cell's capacity (max count over
    cores, 32-granular) is allocated as full 64-slot chunk-halves plus one
    optional 32-slot remainder; remainders are packed pairwise into shared
    halves.  Every gather piece is then <=64 wide at a 32-aligned base, and
    consecutive pieces alternate PE column strips (keeps LDWEIGHTS
    pipelining).  Returns plan dict."""
    rpc = cfg.n // cfg.n_cores
    cnt = np.zeros((cfg.n_cores, JBS, NSB), dtype=np.int64)
    for k in range(cfg.n_cores):
        m = (rows >= k * rpc) & (rows < (k + 1) * rpc)
        r = rows[m] - k * rpc
        cell = (r // BLK) * NSB + (cols[m] // BLK)
        cnt[k] = np.bincount(cell, minlength=JBS * NSB).reshape(JBS, NSB)
    caps = ((cnt.max(axis=0) + 31) // 32 * 32).astype(np.int64)  # [JBS, NSB]
    jbs = []
    for j in range(JBS):
        cj = caps[j]
        nfull = cj // 64                  # full 64-halves per cell
        rem = cj % 64                     # 0 or 32
        tot_half = int(nfull.sum()) + (int((rem > 0).sum()) + 1) // 2
        tot_half += tot_half % 2          # pad to whole chunks
        nch = tot_half // 2
        S = nch * 128
        # allocate: full halves first (per cell, consecutive), then pair
        # remainders into the tail halves
        fullstart = np.zeros(NSB, dtype=np.int64)
        remstart = np.zeros(NSB, dtype=np.int64) - 1
        pieces = [[] for _ in range(nch)]
        h = 0
        for sb in range(NSB):
            fullstart[sb] = h * 64
            for _ in range(int(nfull[sb])):
                s0 = h * 64
                pieces[s0 // 128].append((sb, s0, s0 % 128, 64))
                h += 1
        slot2 = 0  # 0 -> low 32 of current half, 1 -> high 32
        for sb in range(NSB):
            if rem[sb]:
                s0 = h * 64 + slot2 * 32
                remstart[sb] = s0
                pieces[s0 // 128].append((sb, s0, s0 % 128, 32))
                if slot2 == 1:
                    h += 1
                slot2 ^= 1
        jbs.append({
            "caps": cj, "nfull": nfull, "fullstart": fullstart,
            "remstart": remstart, "S": S, "nch": nch, "pieces": pieces,
        })
    nch_tot = sum(jb["nch"] for jb in jbs)
    chunk_base = np.concatenate([[0], np.cumsum([jb["nch"] for jb in jbs])])
    return {"jbs": jbs, "nch_tot": int(nch_tot), "chunk_base": chunk_base,
            "key": tuple(int(x) for x in caps.reshape(-1))}


def _make_tables(e, src, dst, cfg: Cfg):
    """Per-core device tables + shared plan."""
    import ml_dtypes

    n = cfg.n
    rows, cols, w = _entries(e, src, dst, n)
    rpc = n // cfg.n_cores
    plan = _make_plan(rows, cols, cfg)
    jbs = plan["jbs"]
    nch_tot = plan["nch_tot"]
    cbase = plan["chunk_base"]
    S_tot = sum(jb["S"] for jb in jbs)
    s_base = np.concatenate([[0], np.cumsum([jb["S"] for jb in jbs])])

    tables = []
    for k in range(cfg.n_cores):
        m = (rows >= k * rpc) & (rows < (k + 1) * rpc)
        r = rows[m] - k * rpc
        c = cols[m]
        wv = w[:, m]
        jb = r // BLK
        dl = r % BLK
        sb = c // BLK
        sl = c % BLK
        o = np.lexsort((sb, jb))
        jb, dl, sb, sl, wv = jb[o], dl[o], sb[o], sl[o], wv[:, o]
        cell = jb * NSB + sb
        ic = np.arange(len(cell)) - np.searchsorted(cell, cell)

        gmat = np.zeros((128, S_tot), dtype=ml_dtypes.float8_e4m3fn)
        sca = np.zeros((128, nch_tot * 128), dtype=ml_dtypes.float8_e4m3fn)
        w4m = np.zeros((128, nch_tot, H), dtype=np.float32)

        fullstart = np.stack([jbs[j]["fullstart"] for j in range(JBS)])
        remstart = np.stack([jbs[j]["remstart"] for j in range(JBS)])
        nfull64 = np.stack([jbs[j]["nfull"] * 64 for j in range(JBS)])
        slot = np.where(
            ic < nfull64[jb, sb],
            fullstart[jb, sb] + ic,
            remstart[jb, sb] + ic - nfull64[jb, sb],
        )
        gcol = s_base[jb] + slot
        chunk = cbase[jb] + slot // 128
        p = slot % 128
        gmat[sl, gcol] = 1.0
        sca[p, chunk * 128 + dl] = 1.0
        w4m[p, chunk, :] = wv.T

        tables.append({
            "gmat": np.ascontiguousarray(gmat),
            "sca": np.ascontiguousarray(sca),
            "w4m": np.ascontiguousarray(w4m.reshape(128, -1)),
        })
    return tables, plan


def _emulate(tables, plan, xe, cfg: Cfg):
    """Numpy emulation of one iteration y = A @ x using tables + plan."""
    jbs = plan["jbs"]
    cbase = plan["chunk_base"]
    s_base = np.concatenate([[0], np.cumsum([jb["S"] for jb in jbs])])
    out = np.zeros((NCORES, 128, JBS, 64), dtype=np.float32)
    xsb = (
        xe.reshape(NCORES, 128, JBS, 64).transpose(1, 0, 2, 3)
        .reshape(128, NSB, 64).astype(np.float32)
    )
    for k in range(cfg.n_cores):
        t = tables[k]
        gmat = t["gmat"].astype(np.float32)
        sca = t["sca"].astype(np.float32)
        w4m = t["w4m"].reshape(128, -1, H)
        for j in range(JBS):
            acc = np.zeros((128, 64), dtype=np.float32)
            for ci in range(jbs[j]["nch"]):
                cg = cbase[j] + ci
                pg = np.zeros((128, 64), dtype=np.float32)
                for (sb, s0, base, wd) in jbs[j]["pieces"][ci]:
                    g = gmat[:, s_base[j] + s0: s_base[j] + s0 + wd]
                    pg[base:base + wd] = g.T @ xsb[:, sb, :]
                w4 = w4m[:, cg, :]
                xgw = (pg.reshape(128, H, d)
                       * w4[:, :, None]).reshape(128, 64).astype(np.float16)
                s = sca[:, cg * 128:(cg + 1) * 128]
                acc += s.T @ xgw.astype(np.float32)
            out[k, :, j, :] = acc
    return out.transpose(0, 2, 1, 3).reshape(N, 64)


# ------------------------------------------------------------ bass program

_FP32 = mybir.dt.float32
_FP16 = mybir.dt.float16
_FP8 = mybir.dt.float8e4


def _build_program(cfg: Cfg, plan):
    kt = cfg.kt
    jbs = plan["jbs"]
    nch_tot = plan["nch_tot"]
    cbase = plan["chunk_base"]
    s_base = np.concatenate([[0], np.cumsum([jb["S"] for jb in jbs])])
    S_tot = int(s_base[-1])
    nc = bacc.Bacc(
        "TRN2",
        target_bir_lowering=False,
        debug=False,
        num_devices=cfg.n_cores,
    )

    xe0_d = nc.dram_tensor("xe0", [1024, 512], _FP16, kind="ExternalInput").ap()
    x0s_d = nc.dram_tensor("x0s", [128, 512], _FP32, kind="ExternalInput").ap()
    gmat_d = nc.dram_tensor("gmat", [128, S_tot], _FP8,
                            kind="ExternalInput").ap()
    sca_d = nc.dram_tensor("sca", [128, nch_tot * 128], _FP8,
                           kind="ExternalInput").ap()
    w4m_d = nc.dram_tensor("w4m", [128, nch_tot * H], _FP32,
                           kind="ExternalInput").ap()
    out_d = nc.dram_tensor("out", [128, 512], _FP32, kind="ExternalOutput").ap()

    slice_in = nc.dram_tensor("slice_in", [128, 512], _FP16).ap()
    xallE = nc.dram_tensor("xallE", [1024, 512], _FP16,
                           addr_space="Shared").ap()
    warm_in = nc.dram_tensor("warm_in", [128, 8], _FP16).ap()
    warm_out = nc.dram_tensor("warm_out", [1024, 8], _FP16,
                              addr_space="Shared").ap()

    groups = [list(range(cfg.n_cores))]

    with tile.TileContext(nc) as tc:
        with (
            tc.tile_pool(name="tables", bufs=1) as tp,
            tc.tile_pool(name="xgw", bufs=3) as xgwp,
            tc.tile_pool(name="psg", bufs=3, space="PSUM") as pgp,
            tc.tile_pool(name="pso", bufs=2, space="PSUM") as pop,
        ):
            w4m_sb = tp.tile([128, nch_tot, H], _FP32)
            xsb = tp.tile([128, NSB, 64], _FP16)
            xnext = tp.tile([128, JBS * 64], _FP16)
            result = tp.tile([128, JBS * 64], _FP32)
            z128 = tp.tile([128, 128], _FP8)
            gmat_sb = [tp.tile([128, jbs[j]["S"]], _FP8, tag=f"gm{j}",
                               name=f"gmat_sb{j}")
                       for j in range(JBS)]
            sca_sb = [tp.tile([128, jbs[j]["nch"] * 128], _FP8, tag=f"sc{j}",
                              name=f"sca_sb{j}")
                      for j in range(JBS)]

            nc.vector.memset(z128[:], 0.0)
            # warm up ncfw: the first collective pays ~45us of cold cost;
            # hide it behind the table loads
            nc.sync.dma_start(out=warm_in, in_=xe0_d[0:128, 0:8])
            nc.gpsimd.collective_compute(
                "AllGather",
                mybir.AluOpType.bypass,
                replica_groups=groups,
                ins=[warm_in],
                outs=[warm_out],
            )
            # small tables + x first so jb0 compute starts early
            nc.sync.dma_start(
                out=w4m_sb[:].rearrange("p c h -> p (c h)"), in_=w4m_d)
            nc.sync.dma_start(
                out=xsb[:].rearrange("p (k j) f -> p k (j f)", k=NCORES),
                in_=xe0_d.rearrange("(k p) f -> p k f", p=128),
            )
            nc.sync.dma_start(out=result[:], in_=x0s_d)
            for j in range(JBS):
                nc.sync.dma_start(
                    out=gmat_sb[j][:],
                    in_=gmat_d[:, int(s_base[j]):int(s_base[j + 1])])
                nc.sync.dma_start(
                    out=sca_sb[j][:],
                    in_=sca_d[:, int(cbase[j]) * 128:int(cbase[j + 1]) * 128])

            for it in range(1, kt + 1):
                coef = 1.0 / math.factorial(it)
                pout = pop.tile([128, JBS * 64], _FP32, tag="pout")
                # open the accumulation bank: zero matmul writes every byte
                # (rhs is any resident finite data; avoids the xsb reload dep)
                nc.tensor.matmul(
                    pout[:],
                    lhsT=z128[:],
                    rhs=gmat_sb[0][:, 0:512],
                    start=True,
                    stop=False,
                )
                for j in range(JBS):
                    nch_j = jbs[j]["nch"]
                    ngr = -(-nch_j // GRP)
                    for g in range(ngr):
                        c0 = g * GRP
                        c1 = min(nch_j, c0 + GRP)
                        gsz = c1 - c0
                        pg = pgp.tile([128, GRP * 64], _FP32, tag="pg")
                        for ci in range(c0, c1):
                            for (sb, s0, base, wd) in jbs[j]["pieces"][ci]:
                                nc.tensor.matmul(
                                    pg[base:base + wd,
                                       (ci - c0) * 64:(ci - c0 + 1) * 64],
                                    lhsT=gmat_sb[j][:, s0:s0 + wd],
                                    rhs=xsb[:, sb, :],
                                    start=True,
                                    stop=True,
                                    tile_position=(
                                        (0, 96) if base == 96 else None),
                                )
                        xgw = xgwp.tile([128, GRP, 64], _FP16, tag="xgw")
                        pg4 = pg[:, 0:gsz * 64].rearrange(
                            "p (c h f) -> p c h f", c=gsz, h=H)
                        wv = (
                            w4m_sb[:, int(cbase[j]) + c0:
                                   int(cbase[j]) + c1, :]
                            .unsqueeze(3)
                            .to_broadcast([128, gsz, H, d])
                        )
                        nc.vector.tensor_mul(
                            xgw[:, 0:gsz, :].rearrange(
                                "p c (h f) -> p c h f", h=H),
                            pg4, wv)
                        last_mm = (j == JBS - 1 and c1 == nch_j)
                        for ci in range(c0, c1):
                            nc.tensor.matmul(
                                pout[:, j * 64:(j + 1) * 64],
                                lhsT=sca_sb[j][:, ci * 128:(ci + 1) * 128],
                                rhs=xgw[:, ci - c0, :],
                                start=False,
                                stop=(last_mm and ci == c1 - 1),
                            )
                # ---- evacuate + Taylor accumulate
                nc.scalar.copy(xnext[:], pout[:])
                nc.vector.scalar_tensor_tensor(
                    result[:],
                    pout[:],
                    coef,
                    result[:],
                    op0=mybir.AluOpType.mult,
                    op1=mybir.AluOpType.add,
                )
                if it < kt:
                    nc.sync.dma_start(out=slice_in, in_=xnext[:])
                    nc.gpsimd.collective_compute(
                        "AllGather",
                        mybir.AluOpType.bypass,
                        replica_groups=groups,
                        ins=[slice_in],
                        outs=[xallE],
                    )
                    nc.sync.dma_start(
                        out=xsb[:].rearrange(
                            "p (k j) f -> p k (j f)", k=NCORES),
                        in_=xallE.rearrange("(k p) f -> p k f", p=128),
                    )

            nc.sync.dma_start(out=out_d, in_=result[:])

    nc.compile()
    return nc


# ------------------------------------------------------------------ driver

_CACHE = {}


def _get_program(cfg: Cfg, plan):
    key = (cfg, plan["key"])
    if key not in _CACHE:
        _CACHE[key] = _build_program(cfg, plan)
    return _CACHE[key]


def _prep_x(h):
    """h [N, D] -> x0 node-major [N, D] (head-interleaved feats)."""
    return np.ascontiguousarray(
        h.reshape(H, N, d).transpose(1, 0, 2).reshape(N, D))


def _to_exchange(x0):
    """node-major [8192, 64] -> exchange layout [1024, 512]."""
    return np.ascontiguousarray(
        x0.reshape(NCORES, JBS, 128, 64).transpose(0, 2, 1, 3)
        .reshape(1024, 512))


def run(h, e, src, dst, cfg: Cfg = Cfg(), trace: bool = False):
    h = np.asarray(h, dtype=np.float32)
    e = np.asarray(e, dtype=np.float32)
    src = np.asarray(src)
    dst = np.asarray(dst)
    assert h.shape == (cfg.n, D) and e.shape == (H, E)

    tables, plan = _make_tables(e, src, dst, cfg)
    x0 = _prep_x(h)
    xe0 = _to_exchange(x0).astype(np.float16)
    in_maps = []
    for k in range(cfg.n_cores):
        x0s = np.ascontiguousarray(
            x0[k * 1024:(k + 1) * 1024]
            .reshape(JBS, 128, 64).transpose(1, 0, 2).reshape(128, 512))
        t = tables[k]
        in_maps.append(
            {
                "xe0": xe0,
                "x0s": x0s,
                "gmat": t["gmat"],
                "sca": t["sca"],
                "w4m": t["w4m"],
            }
        )
    nc = _get_program(cfg, plan)
    res = run_bass_kernel_spmd(
        nc, in_maps, list(range(cfg.n_cores)), trace=trace)
    out = np.stack([res.results[k]["out"] for k in range(cfg.n_cores)])
    out = (out.reshape(NCORES, 128, JBS, 64).transpose(0, 2, 1, 3)
           .reshape(N, 64))
    out = np.ascontiguousarray(
        out.reshape(N, H, d).transpose(1, 0, 2)).reshape(N, D)
    return out, res


def kernel(h, e, src, dst):
    out, _ = run(h, e, src, dst)
    return out


# revision 20
# speedup vs baseline: 2.1669x; 1.0431x over previous
"""Trainium2 Bass kernel for nn_LinearDiffusion (truncated Taylor expm(a) @ x).

Math: a = row-normalized symmetric scatter of per-head edge weights onto an
(H, N, N) zero tensor; result = sum_{i=0..6} a^i x / i! with x = h reshaped
per-head.

Strategy (8 NeuronCores, one chip) — v5, TensorE-gather, no spill:
  * x (8192 x 64 fp16, all heads together) lives in SBUF on every core; the
    per-edge gather x[src] is computed by TensorE one-hot matmuls from the
    SBUF-resident copy (the v1 bottleneck was per-edge DMA gather on GpSimd).
  * Core k owns dst rows [k*1024, (k+1)*1024) = 8 blocks of 128.  Edges are
    binned per (dst block jb, src block sb) cell.  Cell capacities are
    data-adaptive: cap = max-over-cores count, rounded up to 32 (the PE
    column-group granularity), so ALL edges fit — no overflow path.  The
    cell grid (shared by all cores; only table data differs) tiles each dst
    block's slot space into 128-slot chunks:
      - gather:  per piece (cell range clipped to chunk + base-alignment
                 rules), one matmul  psum[base:base+w, chunk] =
                 gmat_cols^T @ xsb[:, sb, :]   (gmat: fp8 one-hot of
                 src_local, zero-padded; every PSUM byte written, start=True)
      - weights: one DVE multiply per 8-chunk PSUM bank with the per-head
                 w4 table (broadcast over the 16 feats of each head), fp16 out
      - scatter: per chunk, one matmul into the iteration's output bank
                 pout[:, jb*64:+64] += sca[:, chunk]^T @ xgw  (sca: fp8
                 one-hot of dst_local; PSUM accumulation opened by a
                 full-bank zero matmul — hw clears has_written per element)
  * Between iterations: AllGather of the fp16 x slices (128 KB/rank) and a
    single strided DMA reload of xsb.  Output accumulates in fp32.
"""

import math
import os
from dataclasses import dataclass

import numpy as np

# Small AllGathers hit the RDH algorithm; try mesh (set before NRT loads).
os.environ.setdefault("NEURON_RT_DBG_RDH_CC", "0")

import concourse.bass as bass  # noqa: F401  (kept for callers)
import concourse.tile as tile
from concourse import bacc, mybir
from concourse.bass_utils import run_bass_kernel_spmd

# ----------------------------------------------------------------- config

N, H, E, D = 8192, 4, 131072, 64
d = D // H
NCORES = 8
BLK = 128                  # dst-block size == PSUM partition width
NSB = N // BLK             # src blocks (64)
JBS = N // NCORES // BLK   # dst blocks per core (8)
K_TAYLOR = 6
GRP = 8                    # chunks per PSUM gather bank


@dataclass(frozen=True)
class Cfg:
    n: int = N
    n_cores: int = NCORES
    kt: int = K_TAYLOR  # Taylor depth (debug)


# ----------------------------------------------------------- preprocessing


def _entries(e, src, dst, n):
    """Unique symmetric entries with 'last write wins' duplicate semantics,
    matching jax's .at[].set() on CPU. Returns (rows, cols, w[H, nnz])."""
    src = src.astype(np.int64)
    dst = dst.astype(np.int64)
    n_edges = len(src)
    keys = np.concatenate([src * n + dst, dst * n + src])
    eid = np.concatenate([np.arange(n_edges), np.arange(n_edges)])
    order = np.arange(2 * n_edges)
    perm = np.lexsort((-order, keys))
    k_sorted = keys[perm]
    first = np.ones(len(k_sorted), dtype=bool)
    first[1:] = k_sorted[1:] != k_sorted[:-1]
    win = perm[first]
    ukeys = k_sorted[first]
    rows = (ukeys // n).astype(np.int64)
    cols = (ukeys % n).astype(np.int64)
    weids = eid[win]
    vals = e[:, weids].astype(np.float64)  # (H, nnz)
    nheads = e.shape[0]
    rowsum = np.zeros((nheads, n), dtype=np.float64)
    for hh in range(nheads):
        rowsum[hh] = np.bincount(rows, weights=vals[hh], minlength=n)
    w = (vals / rowsum[:, rows]).astype(np.float32)
    return rows, cols, w


def _make_plan(rows, cols, cfg: Cfg):
    """Shared (cross-core) cell grid.  Each cell's capacity (max count over
    cores, 32-granular) is allocated as full 64-slot chunk-halves plus one
    optional 32-slot remainder; remainders are packed pairwise into shared
    halves.  Every gather piece is then <=64 wide at a 32-aligned base, and
    consecutive pieces alternate PE column strips (keeps LDWEIGHTS
    pipelining).  Returns plan dict."""
    rpc = cfg.n // cfg.n_cores
    cnt = np.zeros((cfg.n_cores, JBS, NSB), dtype=np.int64)
    for k in range(cfg.n_cores):
        m = (rows >= k * rpc) & (rows < (k + 1) * rpc)
        r = rows[m] - k * rpc
        cell = (r // BLK) * NSB + (cols[m] // BLK)
        cnt[k] = np.bincount(cell, minlength=JBS * NSB).reshape(JBS, NSB)
    caps = ((cnt.max(axis=0) + 31) // 32 * 32).astype(np.int64)  # [JBS, NSB]
    jbs = []
    for j in range(JBS):
        cj = caps[j]
        nfull = cj // 64                  # full 64-halves per cell
        rem = cj % 64                     # 0 or 32
        tot_half = int(nfull.sum()) + (int((rem > 0).sum()) + 1) // 2
        tot_half += tot_half % 2          # pad to whole chunks
        nch = tot_half // 2
        S = nch * 128
        # allocate: full halves first (per cell, consecutive), then pair
        # remainders into the tail halves
        fullstart = np.zeros(NSB, dtype=np.int64)
        remstart = np.zeros(NSB, dtype=np.int64) - 1
        pieces = [[] for _ in range(nch)]
        h = 0
        for sb in range(NSB):
            fullstart[sb] = h * 64
            for _ in range(int(nfull[sb])):
                s0 = h * 64
                pieces[s0 // 128].append((sb, s0, s0 % 128, 64))
                h += 1
        slot2 = 0  # 0 -> low 32 of current half, 1 -> high 32
        for sb in range(NSB):
            if rem[sb]:
                s0 = h * 64 + slot2 * 32
                remstart[sb] = s0
                pieces[s0 // 128].append((sb, s0, s0 % 128, 32))
                if slot2 == 1:
                    h += 1
                slot2 ^= 1
        jbs.append({
            "caps": cj, "nfull": nfull, "fullstart": fullstart,
            "remstart": remstart, "S": S, "nch": nch, "pieces": pieces,
        })
    nch_tot = sum(jb["nch"] for jb in jbs)
    chunk_base = np.concatenate([[0], np.cumsum([jb["nch"] for jb in jbs])])
    return {"jbs": jbs, "nch_tot": int(nch_tot), "chunk_base": chunk_base,
            "key": tuple(int(x) for x in caps.reshape(-1))}


def _make_tables(e, src, dst, cfg: Cfg):
    """Per-core device tables + shared plan."""
    import ml_dtypes

    n = cfg.n
    rows, cols, w = _entries(e, src, dst, n)
    rpc = n // cfg.n_cores
    plan = _make_plan(rows, cols, cfg)
    jbs = plan["jbs"]
    nch_tot = plan["nch_tot"]
    cbase = plan["chunk_base"]
    S_tot = sum(jb["S"] for jb in jbs)
    s_base = np.concatenate([[0], np.cumsum([jb["S"] for jb in jbs])])

    tables = []
    for k in range(cfg.n_cores):
        m = (rows >= k * rpc) & (rows < (k + 1) * rpc)
        r = rows[m] - k * rpc
        c = cols[m]
        wv = w[:, m]
        jb = r // BLK
        dl = r % BLK
        sb = c // BLK
        sl = c % BLK
        o = np.lexsort((sb, jb))
        jb, dl, sb, sl, wv = jb[o], dl[o], sb[o], sl[o], wv[:, o]
        cell = jb * NSB + sb
        ic = np.arange(len(cell)) - np.searchsorted(cell, cell)

        gmat = np.zeros((128, S_tot), dtype=ml_dtypes.float8_e4m3fn)
        sca = np.zeros((128, nch_tot * 128), dtype=ml_dtypes.float8_e4m3fn)
        w4m = np.zeros((128, nch_tot, H), dtype=np.float32)

        fullstart = np.stack([jbs[j]["fullstart"] for j in range(JBS)])
        remstart = np.stack([jbs[j]["remstart"] for j in range(JBS)])
        nfull64 = np.stack([jbs[j]["nfull"] * 64 for j in range(JBS)])
        slot = np.where(
            ic < nfull64[jb, sb],
            fullstart[jb, sb] + ic,
            remstart[jb, sb] + ic - nfull64[jb, sb],
        )
        gcol = s_base[jb] + slot
        chunk = cbase[jb] + slot // 128
        p = slot % 128
        gmat[sl, gcol] = 1.0
        sca[p, chunk * 128 + dl] = 1.0
        w4m[p, chunk, :] = wv.T

        tables.append({
            "gmat": np.ascontiguousarray(gmat),
            "sca": np.ascontiguousarray(sca),
            "w4m": np.ascontiguousarray(w4m.reshape(128, -1)),
        })
    return tables, plan


def _emulate(tables, plan, xe, cfg: Cfg):
    """Numpy emulation of one iteration y = A @ x using tables + plan."""
    jbs = plan["jbs"]
    cbase = plan["chunk_base"]
    s_base = np.concatenate([[0], np.cumsum([jb["S"] for jb in jbs])])
    out = np.zeros((NCORES, 128, JBS, 64), dtype=np.float32)
    xsb = (
        xe.reshape(NCORES, 128, JBS, 64).transpose(1, 0, 2, 3)
        .reshape(128, NSB, 64).astype(np.float32)
    )
    for k in range(cfg.n_cores):
        t = tables[k]
        gmat = t["gmat"].astype(np.float32)
        sca = t["sca"].astype(np.float32)
        w4m = t["w4m"].reshape(128, -1, H)
        for j in range(JBS):
            acc = np.zeros((128, 64), dtype=np.float32)
            for ci in range(jbs[j]["nch"]):
                cg = cbase[j] + ci
                pg = np.zeros((128, 64), dtype=np.float32)
                for (sb, s0, base, wd) in jbs[j]["pieces"][ci]:
                    g = gmat[:, s_base[j] + s0: s_base[j] + s0 + wd]
                    pg[base:base + wd] = g.T @ xsb[:, sb, :]
                w4 = w4m[:, cg, :]
                xgw = (pg.reshape(128, H, d)
                       * w4[:, :, None]).reshape(128, 64).astype(np.float16)
                s = sca[:, cg * 128:(cg + 1) * 128]
                acc += s.T @ xgw.astype(np.float32)
            out[k, :, j, :] = acc
    return out.transpose(0, 2, 1, 3).reshape(N, 64)


# ------------------------------------------------------------ bass program

_FP32 = mybir.dt.float32
_FP16 = mybir.dt.float16
_FP8 = mybir.dt.float8e4


def _build_program(cfg: Cfg, plan):
    kt = cfg.kt
    jbs = plan["jbs"]
    nch_tot = plan["nch_tot"]
    cbase = plan["chunk_base"]
    s_base = np.concatenate([[0], np.cumsum([jb["S"] for jb in jbs])])
    S_tot = int(s_base[-1])
    nc = bacc.Bacc(
        "TRN2",
        target_bir_lowering=False,
        debug=False,
        num_devices=cfg.n_cores,
    )

    xe0_d = nc.dram_tensor("xe0", [1024, 512], _FP16, kind="ExternalInput").ap()
    x0s_d = nc.dram_tensor("x0s", [128, 512], _FP32, kind="ExternalInput").ap()
    gmat_d = nc.dram_tensor("gmat", [128, S_tot], _FP8,
                            kind="ExternalInput").ap()
    sca_d = nc.dram_tensor("sca", [128, nch_tot * 128], _FP8,
                           kind="ExternalInput").ap()
    w4m_d = nc.dram_tensor("w4m", [128, nch_tot * H], _FP32,
                           kind="ExternalInput").ap()
    out_d = nc.dram_tensor("out", [128, 512], _FP32, kind="ExternalOutput").ap()

    slice_in = nc.dram_tensor("slice_in", [128, 512], _FP16).ap()
    xallE = nc.dram_tensor("xallE", [1024, 512], _FP16,
                           addr_space="Shared").ap()
    warm_in = nc.dram_tensor("warm_in", [128, 512], _FP16).ap()
    warm_out = nc.dram_tensor("warm_out", [1024, 512], _FP16,
                              addr_space="Shared").ap()

    groups = [list(range(cfg.n_cores))]

    with tile.TileContext(nc) as tc:
        with (
            tc.tile_pool(name="tables", bufs=1) as tp,
            tc.tile_pool(name="xgw", bufs=3) as xgwp,
            tc.tile_pool(name="psg", bufs=3, space="PSUM") as pgp,
            tc.tile_pool(name="pso", bufs=2, space="PSUM") as pop,
        ):
            w4m_sb = tp.tile([128, nch_tot, H], _FP32)
            xsb = tp.tile([128, NSB, 64], _FP16)
            xnext = tp.tile([128, JBS * 64], _FP16)
            result = tp.tile([128, JBS * 64], _FP32)
            z128 = tp.tile([128, 128], _FP8)
            gmat_sb = [tp.tile([128, jbs[j]["S"]], _FP8, tag=f"gm{j}",
                               name=f"gmat_sb{j}")
                       for j in range(JBS)]
            sca_sb = [tp.tile([128, jbs[j]["nch"] * 128], _FP8, tag=f"sc{j}",
                              name=f"sca_sb{j}")
                      for j in range(JBS)]

            nc.vector.memset(z128[:], 0.0)
            # warm up ncfw: the first collective pays ~45us of cold cost;
            # hide it behind the table loads
            nc.sync.dma_start(out=warm_in, in_=xe0_d[0:128, :])
            nc.gpsimd.collective_compute(
                "AllGather",
                mybir.AluOpType.bypass,
                replica_groups=groups,
                ins=[warm_in],
                outs=[warm_out],
            )
            # small tables + x first so jb0 compute starts early
            nc.sync.dma_start(
                out=w4m_sb[:].rearrange("p c h -> p (c h)"), in_=w4m_d)
            nc.sync.dma_start(
                out=xsb[:].rearrange("p (k j) f -> p k (j f)", k=NCORES),
                in_=xe0_d.rearrange("(k p) f -> p k f", p=128),
            )
            nc.sync.dma_start(out=result[:], in_=x0s_d)
            for j in range(JBS):
                nc.sync.dma_start(
                    out=gmat_sb[j][:],
                    in_=gmat_d[:, int(s_base[j]):int(s_base[j + 1])])
                nc.sync.dma_start(
                    out=sca_sb[j][:],
                    in_=sca_d[:, int(cbase[j]) * 128:int(cbase[j + 1]) * 128])

            for it in range(1, kt + 1):
                coef = 1.0 / math.factorial(it)
                pout = pop.tile([128, JBS * 64], _FP32, tag="pout")
                # open the accumulation bank: zero matmul writes every byte
                # (rhs is any resident finite data; avoids the xsb reload dep)
                nc.tensor.matmul(
                    pout[:],
                    lhsT=z128[:],
                    rhs=gmat_sb[0][:, 0:512],
                    start=True,
                    stop=False,
                )
                for j in range(JBS):
                    nch_j = jbs[j]["nch"]
                    ngr = -(-nch_j // GRP)
                    for g in range(ngr):
                        c0 = g * GRP
                        c1 = min(nch_j, c0 + GRP)
                        gsz = c1 - c0
                        pg = pgp.tile([128, GRP * 64], _FP32, tag="pg")
                        for ci in range(c0, c1):
                            for (sb, s0, base, wd) in jbs[j]["pieces"][ci]:
                                nc.tensor.matmul(
                                    pg[base:base + wd,
                                       (ci - c0) * 64:(ci - c0 + 1) * 64],
                                    lhsT=gmat_sb[j][:, s0:s0 + wd],
                                    rhs=xsb[:, sb, :],
                                    start=True,
                                    stop=True,
                                    tile_position=(
                                        (0, 96) if base == 96 else None),
                                )
                        xgw = xgwp.tile([128, GRP, 64], _FP16, tag="xgw")
                        pg4 = pg[:, 0:gsz * 64].rearrange(
                            "p (c h f) -> p c h f", c=gsz, h=H)
                        wv = (
                            w4m_sb[:, int(cbase[j]) + c0:
                                   int(cbase[j]) + c1, :]
                            .unsqueeze(3)
                            .to_broadcast([128, gsz, H, d])
                        )
                        nc.vector.tensor_mul(
                            xgw[:, 0:gsz, :].rearrange(
                                "p c (h f) -> p c h f", h=H),
                            pg4, wv)
                        last_mm = (j == JBS - 1 and c1 == nch_j)
                        for ci in range(c0, c1):
                            nc.tensor.matmul(
                                pout[:, j * 64:(j + 1) * 64],
                                lhsT=sca_sb[j][:, ci * 128:(ci + 1) * 128],
                                rhs=xgw[:, ci - c0, :],
                                start=False,
                                stop=(last_mm and ci == c1 - 1),
                            )
                # ---- evacuate + Taylor accumulate
                nc.scalar.copy(xnext[:], pout[:])
                nc.vector.scalar_tensor_tensor(
                    result[:],
                    pout[:],
                    coef,
                    result[:],
                    op0=mybir.AluOpType.mult,
                    op1=mybir.AluOpType.add,
                )
                if it < kt:
                    nc.sync.dma_start(out=slice_in, in_=xnext[:])
                    nc.gpsimd.collective_compute(
                        "AllGather",
                        mybir.AluOpType.bypass,
                        replica_groups=groups,
                        ins=[slice_in],
                        outs=[xallE],
                    )
                    nc.sync.dma_start(
                        out=xsb[:].rearrange(
                            "p (k j) f -> p k (j f)", k=NCORES),
                        in_=xallE.rearrange("(k p) f -> p k f", p=128),
                    )

            nc.sync.dma_start(out=out_d, in_=result[:])

    nc.compile()
    return nc


# ------------------------------------------------------------------ driver

_CACHE = {}


def _get_program(cfg: Cfg, plan):
    key = (cfg, plan["key"])
    if key not in _CACHE:
        _CACHE[key] = _build_program(cfg, plan)
    return _CACHE[key]


def _prep_x(h):
    """h [N, D] -> x0 node-major [N, D] (head-interleaved feats)."""
    return np.ascontiguousarray(
        h.reshape(H, N, d).transpose(1, 0, 2).reshape(N, D))


def _to_exchange(x0):
    """node-major [8192, 64] -> exchange layout [1024, 512]."""
    return np.ascontiguousarray(
        x0.reshape(NCORES, JBS, 128, 64).transpose(0, 2, 1, 3)
        .reshape(1024, 512))


def run(h, e, src, dst, cfg: Cfg = Cfg(), trace: bool = False):
    h = np.asarray(h, dtype=np.float32)
    e = np.asarray(e, dtype=np.float32)
    src = np.asarray(src)
    dst = np.asarray(dst)
    assert h.shape == (cfg.n, D) and e.shape == (H, E)

    tables, plan = _make_tables(e, src, dst, cfg)
    x0 = _prep_x(h)
    xe0 = _to_exchange(x0).astype(np.float16)
    in_maps = []
    for k in range(cfg.n_cores):
        x0s = np.ascontiguousarray(
            x0[k * 1024:(k + 1) * 1024]
            .reshape(JBS, 128, 64).transpose(1, 0, 2).reshape(128, 512))
        t = tables[k]
        in_maps.append(
            {
                "xe0": xe0,
                "x0s": x0s,
                "gmat": t["gmat"],
                "sca": t["sca"],
                "w4m": t["w4m"],
            }
        )
    nc = _get_program(cfg, plan)
    res = run_bass_kernel_spmd(
        nc, in_maps, list(range(cfg.n_cores)), trace=trace)
    out = np.stack([res.results[k]["out"] for k in range(cfg.n_cores)])
    out = (out.reshape(NCORES, 128, JBS, 64).transpose(0, 2, 1, 3)
           .reshape(N, 64))
    out = np.ascontiguousarray(
        out.reshape(N, H, d).transpose(1, 0, 2)).reshape(N, D)
    return out, res


def kernel(h, e, src, dst):
    out, _ = run(h, e, src, dst)
    return out


# revision 21
# speedup vs baseline: 2.1907x; 1.0110x over previous
"""Trainium2 Bass kernel for nn_LinearDiffusion (truncated Taylor expm(a) @ x).

Math: a = row-normalized symmetric scatter of per-head edge weights onto an
(H, N, N) zero tensor; result = sum_{i=0..6} a^i x / i! with x = h reshaped
per-head.

Strategy (8 NeuronCores, one chip) — v5, TensorE-gather, no spill:
  * x (8192 x 64 fp16, all heads together) lives in SBUF on every core; the
    per-edge gather x[src] is computed by TensorE one-hot matmuls from the
    SBUF-resident copy (the v1 bottleneck was per-edge DMA gather on GpSimd).
  * Core k owns dst rows [k*1024, (k+1)*1024) = 8 blocks of 128.  Edges are
    binned per (dst block jb, src block sb) cell.  Cell capacities are
    data-adaptive: cap = max-over-cores count, rounded up to 32 (the PE
    column-group granularity), so ALL edges fit — no overflow path.  The
    cell grid (shared by all cores; only table data differs) tiles each dst
    block's slot space into 128-slot chunks:
      - gather:  per piece (cell range clipped to chunk + base-alignment
                 rules), one matmul  psum[base:base+w, chunk] =
                 gmat_cols^T @ xsb[:, sb, :]   (gmat: fp8 one-hot of
                 src_local, zero-padded; every PSUM byte written, start=True)
      - weights: one DVE multiply per 8-chunk PSUM bank with the per-head
                 w4 table (broadcast over the 16 feats of each head), fp16 out
      - scatter: per chunk, one matmul into the iteration's output bank
                 pout[:, jb*64:+64] += sca[:, chunk]^T @ xgw  (sca: fp8
                 one-hot of dst_local; PSUM accumulation opened by a
                 full-bank zero matmul — hw clears has_written per element)
  * Between iterations: AllGather of the fp16 x slices (128 KB/rank) and a
    single strided DMA reload of xsb.  Output accumulates in fp32.
"""

import math
import os
from dataclasses import dataclass

import numpy as np

# Small AllGathers hit the RDH algorithm; try mesh (set before NRT loads).
os.environ.setdefault("NEURON_RT_DBG_RDH_CC", "0")

import concourse.bass as bass  # noqa: F401  (kept for callers)
import concourse.tile as tile
from concourse import bacc, mybir
from concourse.bass_utils import run_bass_kernel_spmd

# ----------------------------------------------------------------- config

N, H, E, D = 8192, 4, 131072, 64
d = D // H
NCORES = 8
BLK = 128                  # dst-block size == PSUM partition width
NSB = N // BLK             # src blocks (64)
JBS = N // NCORES // BLK   # dst blocks per core (8)
K_TAYLOR = 6
GRP = 8                    # chunks per PSUM gather bank


@dataclass(frozen=True)
class Cfg:
    n: int = N
    n_cores: int = NCORES
    kt: int = K_TAYLOR  # Taylor depth (debug)


# ----------------------------------------------------------- preprocessing


def _entries(e, src, dst, n):
    """Unique symmetric entries with 'last write wins' duplicate semantics,
    matching jax's .at[].set() on CPU. Returns (rows, cols, w[H, nnz])."""
    src = src.astype(np.int64)
    dst = dst.astype(np.int64)
    n_edges = len(src)
    keys = np.concatenate([src * n + dst, dst * n + src])
    eid = np.concatenate([np.arange(n_edges), np.arange(n_edges)])
    order = np.arange(2 * n_edges)
    perm = np.lexsort((-order, keys))
    k_sorted = keys[perm]
    first = np.ones(len(k_sorted), dtype=bool)
    first[1:] = k_sorted[1:] != k_sorted[:-1]
    win = perm[first]
    ukeys = k_sorted[first]
    rows = (ukeys // n).astype(np.int64)
    cols = (ukeys % n).astype(np.int64)
    weids = eid[win]
    vals = e[:, weids].astype(np.float64)  # (H, nnz)
    nheads = e.shape[0]
    rowsum = np.zeros((nheads, n), dtype=np.float64)
    for hh in range(nheads):
        rowsum[hh] = np.bincount(rows, weights=vals[hh], minlength=n)
    w = (vals / rowsum[:, rows]).astype(np.float32)
    return rows, cols, w


def _make_plan(rows, cols, cfg: Cfg):
    """Shared (cross-core) cell grid.  Each cell's capacity (max count over
    cores, 32-granular) is allocated as full 64-slot chunk-halves plus one
    optional 32-slot remainder; remainders are packed pairwise into shared
    halves.  Every gather piece is then <=64 wide at a 32-aligned base, and
    consecutive pieces alternate PE column strips (keeps LDWEIGHTS
    pipelining).  Returns plan dict."""
    rpc = cfg.n // cfg.n_cores
    cnt = np.zeros((cfg.n_cores, JBS, NSB), dtype=np.int64)
    for k in range(cfg.n_cores):
        m = (rows >= k * rpc) & (rows < (k + 1) * rpc)
        r = rows[m] - k * rpc
        cell = (r // BLK) * NSB + (cols[m] // BLK)
        cnt[k] = np.bincount(cell, minlength=JBS * NSB).reshape(JBS, NSB)
    caps = ((cnt.max(axis=0) + 31) // 32 * 32).astype(np.int64)  # [JBS, NSB]
    jbs = []
    for j in range(JBS):
        cj = caps[j]
        nfull = cj // 64                  # full 64-halves per cell
        rem = cj % 64                     # 0 or 32
        tot_half = int(nfull.sum()) + (int((rem > 0).sum()) + 1) // 2
        tot_half += tot_half % 2          # pad to whole chunks
        nch = tot_half // 2
        S = nch * 128
        # allocate: full halves first (per cell, consecutive), then pair
        # remainders into the tail halves
        fullstart = np.zeros(NSB, dtype=np.int64)
        remstart = np.zeros(NSB, dtype=np.int64) - 1
        pieces = [[] for _ in range(nch)]
        h = 0
        for sb in range(NSB):
            fullstart[sb] = h * 64
            for _ in range(int(nfull[sb])):
                s0 = h * 64
                pieces[s0 // 128].append((sb, s0, s0 % 128, 64))
                h += 1
        slot2 = 0  # 0 -> low 32 of current half, 1 -> high 32
        for sb in range(NSB):
            if rem[sb]:
                s0 = h * 64 + slot2 * 32
                remstart[sb] = s0
                pieces[s0 // 128].append((sb, s0, s0 % 128, 32))
                if slot2 == 1:
                    h += 1
                slot2 ^= 1
        jbs.append({
            "caps": cj, "nfull": nfull, "fullstart": fullstart,
            "remstart": remstart, "S": S, "nch": nch, "pieces": pieces,
        })
    nch_tot = sum(jb["nch"] for jb in jbs)
    chunk_base = np.concatenate([[0], np.cumsum([jb["nch"] for jb in jbs])])
    return {"jbs": jbs, "nch_tot": int(nch_tot), "chunk_base": chunk_base,
            "key": tuple(int(x) for x in caps.reshape(-1))}


def _make_tables(e, src, dst, cfg: Cfg):
    """Per-core device tables + shared plan."""
    import ml_dtypes

    n = cfg.n
    rows, cols, w = _entries(e, src, dst, n)
    rpc = n // cfg.n_cores
    plan = _make_plan(rows, cols, cfg)
    jbs = plan["jbs"]
    nch_tot = plan["nch_tot"]
    cbase = plan["chunk_base"]
    S_tot = sum(jb["S"] for jb in jbs)
    s_base = np.concatenate([[0], np.cumsum([jb["S"] for jb in jbs])])

    tables = []
    for k in range(cfg.n_cores):
        m = (rows >= k * rpc) & (rows < (k + 1) * rpc)
        r = rows[m] - k * rpc
        c = cols[m]
        wv = w[:, m]
        jb = r // BLK
        dl = r % BLK
        sb = c // BLK
        sl = c % BLK
        o = np.lexsort((sb, jb))
        jb, dl, sb, sl, wv = jb[o], dl[o], sb[o], sl[o], wv[:, o]
        cell = jb * NSB + sb
        ic = np.arange(len(cell)) - np.searchsorted(cell, cell)

        gmat = np.zeros((128, S_tot), dtype=ml_dtypes.float8_e4m3fn)
        sca = np.zeros((128, nch_tot * 128), dtype=ml_dtypes.float8_e4m3fn)
        w4m = np.zeros((128, nch_tot, H), dtype=np.float32)

        fullstart = np.stack([jbs[j]["fullstart"] for j in range(JBS)])
        remstart = np.stack([jbs[j]["remstart"] for j in range(JBS)])
        nfull64 = np.stack([jbs[j]["nfull"] * 64 for j in range(JBS)])
        slot = np.where(
            ic < nfull64[jb, sb],
            fullstart[jb, sb] + ic,
            remstart[jb, sb] + ic - nfull64[jb, sb],
        )
        gcol = s_base[jb] + slot
        chunk = cbase[jb] + slot // 128
        p = slot % 128
        gmat[sl, gcol] = 1.0
        sca[p, chunk * 128 + dl] = 1.0
        w4m[p, chunk, :] = wv.T

        tables.append({
            "gmat": np.ascontiguousarray(gmat),
            "sca": np.ascontiguousarray(sca),
            "w4m": np.ascontiguousarray(w4m.reshape(128, -1)),
        })
    return tables, plan


def _emulate(tables, plan, xe, cfg: Cfg):
    """Numpy emulation of one iteration y = A @ x using tables + plan."""
    jbs = plan["jbs"]
    cbase = plan["chunk_base"]
    s_base = np.concatenate([[0], np.cumsum([jb["S"] for jb in jbs])])
    out = np.zeros((NCORES, 128, JBS, 64), dtype=np.float32)
    xsb = (
        xe.reshape(NCORES, 128, JBS, 64).transpose(1, 0, 2, 3)
        .reshape(128, NSB, 64).astype(np.float32)
    )
    for k in range(cfg.n_cores):
        t = tables[k]
        gmat = t["gmat"].astype(np.float32)
        sca = t["sca"].astype(np.float32)
        w4m = t["w4m"].reshape(128, -1, H)
        for j in range(JBS):
            acc = np.zeros((128, 64), dtype=np.float32)
            for ci in range(jbs[j]["nch"]):
                cg = cbase[j] + ci
                pg = np.zeros((128, 64), dtype=np.float32)
                for (sb, s0, base, wd) in jbs[j]["pieces"][ci]:
                    g = gmat[:, s_base[j] + s0: s_base[j] + s0 + wd]
                    pg[base:base + wd] = g.T @ xsb[:, sb, :]
                w4 = w4m[:, cg, :]
                xgw = (pg.reshape(128, H, d)
                       * w4[:, :, None]).reshape(128, 64).astype(np.float16)
                s = sca[:, cg * 128:(cg + 1) * 128]
                acc += s.T @ xgw.astype(np.float32)
            out[k, :, j, :] = acc
    return out.transpose(0, 2, 1, 3).reshape(N, 64)


# ------------------------------------------------------------ bass program

_FP32 = mybir.dt.float32
_FP16 = mybir.dt.float16
_FP8 = mybir.dt.float8e4


def _build_program(cfg: Cfg, plan):
    kt = cfg.kt
    jbs = plan["jbs"]
    nch_tot = plan["nch_tot"]
    cbase = plan["chunk_base"]
    s_base = np.concatenate([[0], np.cumsum([jb["S"] for jb in jbs])])
    S_tot = int(s_base[-1])
    nc = bacc.Bacc(
        "TRN2",
        target_bir_lowering=False,
        debug=False,
        num_devices=cfg.n_cores,
    )

    xe0_d = nc.dram_tensor("xe0", [1024, 512], _FP16, kind="ExternalInput").ap()
    x0s_d = nc.dram_tensor("x0s", [128, 512], _FP32, kind="ExternalInput").ap()
    gmat_d = nc.dram_tensor("gmat", [128, S_tot], _FP8,
                            kind="ExternalInput").ap()
    sca_d = nc.dram_tensor("sca", [128, nch_tot * 128], _FP8,
                           kind="ExternalInput").ap()
    w4m_d = nc.dram_tensor("w4m", [128, nch_tot * H], _FP32,
                           kind="ExternalInput").ap()
    out_d = nc.dram_tensor("out", [128, 512], _FP32, kind="ExternalOutput").ap()

    slice_in = nc.dram_tensor("slice_in", [128, 512], _FP16).ap()
    xallE = nc.dram_tensor("xallE", [1024, 512], _FP16,
                           addr_space="Shared").ap()
    warm_in = nc.dram_tensor("warm_in", [128, 512], _FP16).ap()
    warm_out = nc.dram_tensor("warm_out", [1024, 512], _FP16,
                              addr_space="Shared").ap()

    groups = [list(range(cfg.n_cores))]

    with tile.TileContext(nc) as tc:
        with (
            tc.tile_pool(name="tables", bufs=1) as tp,
            tc.tile_pool(name="xgw", bufs=6) as xgwp,
            tc.tile_pool(name="psg", bufs=5, space="PSUM") as pgp,
            tc.tile_pool(name="pso", bufs=2, space="PSUM") as pop,
        ):
            w4m_sb = tp.tile([128, nch_tot, H], _FP32)
            xsb = tp.tile([128, NSB, 64], _FP16)
            xnext = tp.tile([128, JBS * 64], _FP16)
            result = tp.tile([128, JBS * 64], _FP32)
            z128 = tp.tile([128, 128], _FP8)
            gmat_sb = [tp.tile([128, jbs[j]["S"]], _FP8, tag=f"gm{j}",
                               name=f"gmat_sb{j}")
                       for j in range(JBS)]
            sca_sb = [tp.tile([128, jbs[j]["nch"] * 128], _FP8, tag=f"sc{j}",
                              name=f"sca_sb{j}")
                      for j in range(JBS)]

            nc.vector.memset(z128[:], 0.0)
            # warm up ncfw: the first collective pays ~45us of cold cost;
            # hide it behind the table loads
            nc.sync.dma_start(out=warm_in, in_=xe0_d[0:128, :])
            nc.gpsimd.collective_compute(
                "AllGather",
                mybir.AluOpType.bypass,
                replica_groups=groups,
                ins=[warm_in],
                outs=[warm_out],
            )
            # small tables + x first so jb0 compute starts early
            nc.sync.dma_start(
                out=w4m_sb[:].rearrange("p c h -> p (c h)"), in_=w4m_d)
            nc.sync.dma_start(
                out=xsb[:].rearrange("p (k j) f -> p k (j f)", k=NCORES),
                in_=xe0_d.rearrange("(k p) f -> p k f", p=128),
            )
            nc.sync.dma_start(out=result[:], in_=x0s_d)
            for j in range(JBS):
                nc.sync.dma_start(
                    out=gmat_sb[j][:],
                    in_=gmat_d[:, int(s_base[j]):int(s_base[j + 1])])
                nc.sync.dma_start(
                    out=sca_sb[j][:],
                    in_=sca_d[:, int(cbase[j]) * 128:int(cbase[j + 1]) * 128])

            for it in range(1, kt + 1):
                coef = 1.0 / math.factorial(it)
                pout = pop.tile([128, JBS * 64], _FP32, tag="pout")
                # open the accumulation bank: zero matmul writes every byte
                # (rhs is any resident finite data; avoids the xsb reload dep)
                nc.tensor.matmul(
                    pout[:],
                    lhsT=z128[:],
                    rhs=gmat_sb[0][:, 0:512],
                    start=True,
                    stop=False,
                )
                for j in range(JBS):
                    nch_j = jbs[j]["nch"]
                    ngr = -(-nch_j // GRP)
                    for g in range(ngr):
                        c0 = g * GRP
                        c1 = min(nch_j, c0 + GRP)
                        gsz = c1 - c0
                        pg = pgp.tile([128, GRP * 64], _FP32, tag="pg")
                        for ci in range(c0, c1):
                            for (sb, s0, base, wd) in jbs[j]["pieces"][ci]:
                                nc.tensor.matmul(
                                    pg[base:base + wd,
                                       (ci - c0) * 64:(ci - c0 + 1) * 64],
                                    lhsT=gmat_sb[j][:, s0:s0 + wd],
                                    rhs=xsb[:, sb, :],
                                    start=True,
                                    stop=True,
                                    tile_position=(
                                        (0, 96) if base == 96 else None),
                                )
                        xgw = xgwp.tile([128, GRP, 64], _FP16, tag="xgw")
                        pg4 = pg[:, 0:gsz * 64].rearrange(
                            "p (c h f) -> p c h f", c=gsz, h=H)
                        wv = (
                            w4m_sb[:, int(cbase[j]) + c0:
                                   int(cbase[j]) + c1, :]
                            .unsqueeze(3)
                            .to_broadcast([128, gsz, H, d])
                        )
                        nc.vector.tensor_mul(
                            xgw[:, 0:gsz, :].rearrange(
                                "p c (h f) -> p c h f", h=H),
                            pg4, wv)
                        last_mm = (j == JBS - 1 and c1 == nch_j)
                        for ci in range(c0, c1):
                            nc.tensor.matmul(
                                pout[:, j * 64:(j + 1) * 64],
                                lhsT=sca_sb[j][:, ci * 128:(ci + 1) * 128],
                                rhs=xgw[:, ci - c0, :],
                                start=False,
                                stop=(last_mm and ci == c1 - 1),
                            )
                # ---- evacuate + Taylor accumulate
                nc.scalar.copy(xnext[:], pout[:])
                nc.vector.scalar_tensor_tensor(
                    result[:],
                    pout[:],
                    coef,
                    result[:],
                    op0=mybir.AluOpType.mult,
                    op1=mybir.AluOpType.add,
                )
                if it < kt:
                    nc.sync.dma_start(out=slice_in, in_=xnext[:])
                    nc.gpsimd.collective_compute(
                        "AllGather",
                        mybir.AluOpType.bypass,
                        replica_groups=groups,
                        ins=[slice_in],
                        outs=[xallE],
                    )
                    nc.sync.dma_start(
                        out=xsb[:].rearrange(
                            "p (k j) f -> p k (j f)", k=NCORES),
                        in_=xallE.rearrange("(k p) f -> p k f", p=128),
                    )

            nc.sync.dma_start(out=out_d, in_=result[:])

    nc.compile()
    return nc


# ------------------------------------------------------------------ driver

_CACHE = {}


def _get_program(cfg: Cfg, plan):
    key = (cfg, plan["key"])
    if key not in _CACHE:
        _CACHE[key] = _build_program(cfg, plan)
    return _CACHE[key]


def _prep_x(h):
    """h [N, D] -> x0 node-major [N, D] (head-interleaved feats)."""
    return np.ascontiguousarray(
        h.reshape(H, N, d).transpose(1, 0, 2).reshape(N, D))


def _to_exchange(x0):
    """node-major [8192, 64] -> exchange layout [1024, 512]."""
    return np.ascontiguousarray(
        x0.reshape(NCORES, JBS, 128, 64).transpose(0, 2, 1, 3)
        .reshape(1024, 512))


def run(h, e, src, dst, cfg: Cfg = Cfg(), trace: bool = False):
    h = np.asarray(h, dtype=np.float32)
    e = np.asarray(e, dtype=np.float32)
    src = np.asarray(src)
    dst = np.asarray(dst)
    assert h.shape == (cfg.n, D) and e.shape == (H, E)

    tables, plan = _make_tables(e, src, dst, cfg)
    x0 = _prep_x(h)
    xe0 = _to_exchange(x0).astype(np.float16)
    in_maps = []
    for k in range(cfg.n_cores):
        x0s = np.ascontiguousarray(
            x0[k * 1024:(k + 1) * 1024]
            .reshape(JBS, 128, 64).transpose(1, 0, 2).reshape(128, 512))
        t = tables[k]
        in_maps.append(
            {
                "xe0": xe0,
                "x0s": x0s,
                "gmat": t["gmat"],
                "sca": t["sca"],
                "w4m": t["w4m"],
            }
        )
    nc = _get_program(cfg, plan)
    res = run_bass_kernel_spmd(
        nc, in_maps, list(range(cfg.n_cores)), trace=trace)
    out = np.stack([res.results[k]["out"] for k in range(cfg.n_cores)])
    out = (out.reshape(NCORES, 128, JBS, 64).transpose(0, 2, 1, 3)
           .reshape(N, 64))
    out = np.ascontiguousarray(
        out.reshape(N, H, d).transpose(1, 0, 2)).reshape(N, D)
    return out, res


def kernel(h, e, src, dst):
    out, _ = run(h, e, src, dst)
    return out
